# revision 9
# baseline (speedup 1.0000x reference)
"""Causal self-attention Trainium2 Bass kernel (V4).

Full-input contract: kernel(**inputs) takes the unsharded inputs
(x [8,1024,768], W_attn [768,2304], b_attn [2304], W_proj [768,768],
b_proj [768]) and returns the full output [8,1024,768].

Sharding: data parallel - batch element b runs on NeuronCore b (B=8 =
n_cores), no collectives needed.

V4 changes vs V3 (trace-driven; V3 span 303us, PE 64% cov, ACT-bound
attention with a 6.5us single-partition reciprocal on the critical path):
  - scores land in bf16 PSUM tiles (1 bank instead of 2): per-head sp
    double-buffered + both heads' score MMs adjacent -> row-group
    concurrent (K=64 pairs at rows 0:64 / 64:128), single MM per tk-tile
    (N up to 1024 bf16 moving).
  - avp split into L/R [65,512] banks; L evicts at i=3 overlapping the
    R-half AV stream. PSUM: 4 sp banks + 4 avp banks = 8 exactly.
  - softmax divide: reciprocal_approx_fast straight off the PSUM l-row,
    1/l broadcast via gpsimd partition_broadcast (SBUF only), multiply
    fused into the avp eviction. No PE broadcast MMs, no yS staging copy.
  - bias adds: b_attn(v part) / b_proj partition-broadcast once via
    gpsimd; evictions become DVE tensor_tensor adds. Kills 32 K=1 bias
    MMs (the qk bias stays a per-partition tensor_scalar_add).
  - pb (exp scores) bf16: halves gpsimd affine_select + AV moving SBUF.
"""

import os
import sys

import numpy as np

for _p in ("/opt/trn_rl_repo", "/root/.axon_site/_ro/trn_rl_repo"):
    if os.path.isdir(_p) and _p not in sys.path:
        sys.path.insert(0, _p)
        break

import concourse.bass as bass
import concourse.mybir as mybir
import concourse.tile as tile
from concourse.bass_utils import run_bass_kernel_spmd
from concourse.masks import make_identity

T, C, H = 1024, 768, 12
C3 = 3 * C
NCORES = 8
NT = T // 128    # 8 t-tiles
NC_ = C // 128   # 6 c-tiles
NM = 2 * C // 128  # 12 m-tiles covering q,k output cols
f32 = mybir.dt.float32
f32r = mybir.dt.float32r
bf16 = mybir.dt.bfloat16

EXP = mybir.ActivationFunctionType.Exp


def build_module():
    md = f32r
    nc = bass.Bass()
    x_d = nc.dram_tensor("x", [T, C], md, kind="ExternalInput")
    wa_d = nc.dram_tensor("W_attn", [C, C3], md, kind="ExternalInput")
    ba_d = nc.dram_tensor("b_attn", [1, C3], md, kind="ExternalInput")
    wp_d = nc.dram_tensor("W_proj", [C, C], md, kind="ExternalInput")
    bp_d = nc.dram_tensor("b_proj", [1, C], md, kind="ExternalInput")
    out_d = nc.dram_tensor("out", [T, C], f32, kind="ExternalOutput")

    with tile.TileContext(nc) as tc:
        with tc.tile_pool(name="persist", bufs=1) as P0:
            identf = P0.tile([128, 128], f32, name="identf")
            make_identity(nc, identf[:])
            ident = P0.tile([128, 128], md, name="ident")
            nc.vector.tensor_copy(ident[:], identf[:])
            ones_f = P0.tile([128, 128], f32, name="ones_f")
            nc.vector.memset(ones_f[:], 1.0)
            ones_col = P0.tile([128, H], bf16, name="ones_col")
            nc.vector.tensor_copy(ones_col[:], ones_f[:, 0:H])

            qkT = [P0.tile([128, T], bf16, name=f"qkT{m}") for m in range(NM)]
            vA = [P0.tile([128, 65 * H], bf16, name=f"vA{t}") for t in range(NT)]
            yT = [P0.tile([128, T], md, name=f"yT{c}") for c in range(NC_)]
            ba_sb = P0.tile([1, C], md, name="ba_sb")
            bp_sb = P0.tile([1, C], md, name="bp_sb")
            baB = P0.tile([128, C], md, name="baB")   # b_attn v-part bcast
            bpB = P0.tile([128, C], md, name="bpB")   # b_proj bcast
            wpt = [P0.tile([128, C], md, name=f"wp{c}") for c in range(NC_)]
            bqk = [P0.tile([128, 1], f32, name=f"bqk{m}") for m in range(NM)]

            # ---- phase 1: x load (FIRST DMAs issued) + transpose ----
            with tc.tile_pool(name="sb1", bufs=3) as SB1:
                xT = [SB1.tile([128, T], md, name=f"xT{c}", tag=f"xT{c}", bufs=1)
                      for c in range(NC_)]
                with tc.tile_pool(name="ps1", bufs=1, space="PSUM") as PS1:
                    for j2 in range(2):
                        trs = [PS1.tile([128, 512], md, tag=f"tr{c}", name=f"tr{c}")
                               for c in range(NC_)]
                        for u in range(4):
                            t = 4 * j2 + u
                            xt = SB1.tile([128, C], md, tag="xt", name="xt")
                            nc.sync.dma_start(out=xt[:], in_=x_d[128 * t:128 * (t + 1), :])
                            for c in range(NC_):
                                nc.tensor.transpose(trs[c][:, 128 * u:128 * (u + 1)],
                                                    xt[:, 128 * c:128 * (c + 1)], ident[:])
                        for c in range(NC_):
                            nc.vector.tensor_copy(xT[c][:, 512 * j2:512 * (j2 + 1)],
                                                  trs[c][:])

                # ---- phase 2: v then q^T/k^T ----
                with tc.tile_pool(name="ps12", bufs=2, space="PSUM") as PS12, \
                     tc.tile_pool(name="sb12", bufs=3) as SB12:
                    # v: stationary xT tiles, moving W_v columns
                    wV = [SB12.tile([128, C], md, name=f"wV{c}", tag=f"wV{c}", bufs=1)
                          for c in range(NC_)]
                    for c in range(NC_):
                        nc.sync.dma_start(out=wV[c][:],
                                          in_=wa_d[128 * c:128 * (c + 1), 2 * C:3 * C])
                    nc.sync.dma_start(out=ba_sb[:], in_=ba_d[0:1, 2 * C:3 * C])
                    # one-time bias broadcast (free-dim stride-0 DMA replicate)
                    nc.sync.dma_start(
                        out=baB[:],
                        in_=ba_sb[0:1, :].unsqueeze(1).to_broadcast([1, 128, C]))
                    baB_r = baB.rearrange("p (h e) -> p h e", h=H)
                    for t in range(NT):
                        accv = PS12.tile([128, C], f32, tag="v", name="accv")
                        for c in range(NC_):
                            xcol = xT[c][:, 128 * t:128 * (t + 1)]
                            nc.tensor.matmul(accv[:, 0:512], xcol, wV[c][:, 0:512],
                                             start=(c == 0), stop=(c == NC_ - 1))
                            nc.tensor.matmul(accv[:, 512:C], xcol, wV[c][:, 512:C],
                                             start=(c == 0), stop=(c == NC_ - 1))
                        av = vA[t].rearrange("p (h e) -> p h e", h=H)
                        nc.vector.tensor_copy(
                            av[:, :, 64:65],
                            ones_col.rearrange("p (h o) -> p h o", o=1))
                        # eviction with fused bias add (replaces bias MMs)
                        nc.vector.tensor_tensor(
                            av[:, :, 0:64],
                            accv[:].rearrange("p (h e) -> p h e", h=H),
                            baB_r[:, :, 0:64],
                            mybir.AluOpType.add)

                    # q^T / k^T: W_attn halves batched per c-tile; the k half
                    # reuses the wV slots (freed once the v matmuls finish)
                    for m in range(NM):
                        nc.sync.dma_start(
                            out=bqk[m][:],
                            in_=ba_d.bitcast(f32)[0:1, 128 * m:128 * (m + 1)]
                                .rearrange("a p -> p a"))
                    wAq = [SB12.tile([128, C], md, name=f"wAq{c}", tag=f"wAq{c}",
                                     bufs=1) for c in range(NC_)]
                    for half in range(2):
                        if half == 0:
                            wh = wAq
                        else:
                            wh = [SB12.tile([128, C], md, name=f"wAk{c}",
                                            tag=f"wV{c}", bufs=1)
                                  for c in range(NC_)]
                        for c in range(NC_):
                            nc.sync.dma_start(
                                out=wh[c][:],
                                in_=wa_d[128 * c:128 * (c + 1), C * half:C * (half + 1)])
                        for mm in range(NC_):
                            m = NC_ * half + mm
                            acc = PS12.tile([128, T], f32, tag="qk", name="acc")
                            for c in range(NC_):
                                wa = wh[c][:, 128 * mm:128 * (mm + 1)]
                                for j2 in range(2):
                                    nc.tensor.matmul(
                                        acc[:, 512 * j2:512 * (j2 + 1)],
                                        wa,
                                        xT[c][:, 512 * j2:512 * (j2 + 1)],
                                        start=(c == 0), stop=(c == NC_ - 1),
                                    )
                            # psum -> sbuf(bf16) with per-partition bias add
                            nc.vector.tensor_scalar_add(qkT[m][:], acc[:], bqk[m][:])

            # ---- phase 3: attention (head pairs; pipelined scores->AV) ----
            # W_proj / b_proj loads issued here: sync queue is idle now and
            # phase 4 needs them much later
            nc.sync.dma_start(out=bp_sb[:], in_=bp_d[:])
            nc.sync.dma_start(
                out=bpB[:],
                in_=bp_sb[0:1, :].unsqueeze(1).to_broadcast([1, 128, C]))
            for c in range(NC_):
                nc.sync.dma_start(out=wpt[c][:], in_=wp_d[128 * c:128 * (c + 1), :])
            with tc.tile_pool(name="ps3", bufs=1, space="PSUM") as PS3, \
                 tc.tile_pool(name="sb3", bufs=4) as SB3:
                for hp in range(H // 2):
                    qt = qkT[hp]
                    kt = qkT[NC_ + hp]

                    # per-head-pair state
                    sps = {}        # (i, hs) -> score tile (live window)
                    avs = {}        # (hs, half) -> [65,512] accumulator
                    for hs in range(2):
                        for half in range(2):
                            avs[(hs, half)] = PS3.tile(
                                [65, 512], f32, tag=f"av{hs}{half}", bufs=1,
                                name=f"av{hs}{half}")

                    def emit_score(i):
                        lo = 128 * i
                        for hs in range(2):
                            base = 64 * hs
                            sp = PS3.tile([128, T], f32, tag="s", bufs=2,
                                          name=f"sp{hs}")
                            ktile = kt[base:base + 64, lo:lo + 128]
                            if lo < 512:
                                nc.tensor.matmul(sp[:, lo:512], ktile,
                                                 qt[base:base + 64, lo:512],
                                                 start=True, stop=True)
                                nc.tensor.matmul(sp[:, 512:T], ktile,
                                                 qt[base:base + 64, 512:T],
                                                 start=True, stop=True)
                            else:
                                nc.tensor.matmul(sp[:, lo:T], ktile,
                                                 qt[base:base + 64, lo:T],
                                                 start=True, stop=True)
                            sps[(i, hs)] = sp

                    def emit_exp_sel(i):
                        lo = 128 * i
                        for hs in range(2):
                            sp = sps.pop((i, hs))
                            pb = SB3.tile([128, T], bf16, tag="pb", bufs=6,
                                          name="pb")
                            nc.scalar.activation(pb[:, lo:T], sp[:, lo:T], EXP,
                                                 scale=0.125)
                            # diagonal [128,128] sub-tile: keep iff p <= f
                            nc.gpsimd.affine_select(
                                out=pb[:, lo:lo + 128], in_=pb[:, lo:lo + 128],
                                pattern=[[1, 128]],
                                compare_op=mybir.AluOpType.is_ge, fill=0.0,
                                base=0, channel_multiplier=-1,
                            )
                            sps[("pb", i, hs)] = pb

                    def emit_av(i):
                        lo = 128 * i
                        for hs in range(2):
                            h = 2 * hp + hs
                            pb = sps.pop(("pb", i, hs))
                            vt = vA[i][:, 65 * h:65 * h + 65]
                            if lo < 512:
                                nc.tensor.matmul(avs[(hs, 0)][0:65, lo:512], vt,
                                                 pb[:, lo:512], start=(i == 0),
                                                 stop=(i == 3),
                                                 skip_group_check=True)
                                nc.tensor.matmul(avs[(hs, 1)][0:65, 0:512], vt,
                                                 pb[:, 512:T], start=(i == 0),
                                                 stop=(i == NT - 1),
                                                 skip_group_check=True)
                            else:
                                nc.tensor.matmul(avs[(hs, 1)][0:65, lo - 512:512],
                                                 vt, pb[:, lo:T], start=False,
                                                 stop=(i == NT - 1),
                                                 skip_group_check=True)

                    def emit_norm(half):
                        # l rows -> SBUF (ACT/DVE split), one reciprocal per
                        # half, 1/l replicated across partitions by a
                        # stride-0 DMA, multiply fused into the eviction
                        lp = SB3.tile([33, 512], f32, tag="lp", bufs=2,
                                      name="lp")
                        nc.scalar.copy(lp[0:1, :], avs[(0, half)][64:65, 0:512])
                        nc.vector.tensor_copy(lp[32:33, :],
                                              avs[(1, half)][64:65, 0:512])
                        rl = SB3.tile([33, 512], f32, tag="rl", bufs=2,
                                      name="rl")
                        # rows 1..31 are don't-care garbage; only 0 and 32
                        # carry the two heads' l
                        nc.vector.reciprocal(rl[:], lp[:])
                        for hs in range(2):
                            avp = avs[(hs, half)]
                            base = 64 * hs
                            rlb = SB3.tile([64, 512], f32, tag="rlb", bufs=4,
                                           name="rlb")
                            nc.sync.dma_start(
                                out=rlb[:],
                                in_=rl[32 * hs:32 * hs + 1, :].unsqueeze(1)
                                    .to_broadcast([1, 64, 512]))
                            nc.vector.tensor_tensor(
                                yT[hp][base:base + 64, 512 * half:512 * (half + 1)],
                                avp[0:64, 0:512], rlb[:],
                                mybir.AluOpType.mult)

                    # software-pipelined emission: scores run one tk-tile
                    # ahead of the AV stream; L-half normalization overlaps
                    # the i>=4 AV matmuls
                    emit_score(0)
                    emit_exp_sel(0)
                    for i in range(1, NT):
                        emit_score(i)
                        emit_exp_sel(i)
                        emit_av(i - 1)
                        if i == 5:
                            emit_norm(0)
                    emit_av(NT - 1)
                    emit_norm(1)

            # ---- phase 4: out = y^T.T @ W_proj + b_proj ----
            with tc.tile_pool(name="ps4", bufs=2, space="PSUM") as PS4, \
                 tc.tile_pool(name="sb4", bufs=3) as SB4:
                for t in range(NT):
                    acc = PS4.tile([128, C], f32, tag="pj", name="acc")
                    for c in range(NC_):
                        ycol = yT[c][:, 128 * t:128 * (t + 1)]
                        nc.tensor.matmul(acc[:, 0:512], ycol, wpt[c][:, 0:512],
                                         start=(c == 0), stop=(c == NC_ - 1))
                        nc.tensor.matmul(acc[:, 512:C], ycol, wpt[c][:, 512:C],
                                         start=(c == 0), stop=(c == NC_ - 1))
                    ot = SB4.tile([128, C], f32, tag="ot", bufs=3, name="ot")
                    # eviction with fused bias add (replaces bias MMs)
                    nc.vector.tensor_tensor(ot[:], acc[:], bpB[:],
                                            mybir.AluOpType.add)
                    nc.sync.dma_start(out=out_d[128 * t:128 * (t + 1), :], in_=ot[:])

    return nc


_WAIT_SKIP = {"InstNoOp", "InstEventSemOp", "InstSemaphoreOp",
              "InstCustomDveAnt", "InstPartitionBroadcast",
              "InstPartitionAllReduce"}


def _legalize_waits(nc):
    """walrus's codegen allows limited sync-wait commands per ISA struct
    (e.g. a Matmult's waits all land on the generated LDWEIGHTS struct which
    has one slot). Move excess waits onto same-engine NoOps inserted
    immediately before the instruction - program order on the engine queue
    preserves the synchronization semantics."""
    nfix = 0
    for fn in nc.m.functions:
        for bb in fn.blocks:
            out = []
            for ins in bb.instructions:
                si = ins.sync_info
                if (type(ins).__name__ not in _WAIT_SKIP and si is not None
                        and si.on_wait and len(si.on_wait) > 1):
                    waits = list(si.on_wait)
                    extra, keep = waits[:-1], waits[-1:]
                    for k, w in enumerate(extra):
                        nop = mybir.InstNoOp(name=f"{ins.name}-wf{k}", ins=[], outs=[])
                        nop.engine = ins.engine
                        nop.sync_info = mybir.SyncInfo(on_wait=[w], on_update=[])
                        out.append(nop)
                    ins.sync_info = mybir.SyncInfo(
                        on_wait=keep, on_update=list(si.on_update or []))
                    nfix += 1
                out.append(ins)
            bb.instructions = out
    return nfix


_cached_module = None


def _get_module():
    global _cached_module
    if _cached_module is None:
        nc = build_module()
        _legalize_waits(nc)
        _cached_module = nc
    return _cached_module


def make_in_maps(x, W_attn, b_attn, W_proj, b_proj):
    x = np.asarray(x, dtype=np.float32)
    wa = np.ascontiguousarray(np.asarray(W_attn, dtype=np.float32))
    ba = np.ascontiguousarray(np.asarray(b_attn, dtype=np.float32).reshape(1, C3))
    wp = np.ascontiguousarray(np.asarray(W_proj, dtype=np.float32))
    bp = np.ascontiguousarray(np.asarray(b_proj, dtype=np.float32).reshape(1, C))
    return [
        dict(x=np.ascontiguousarray(x[b]), W_attn=wa, b_attn=ba, W_proj=wp, b_proj=bp)
        for b in range(x.shape[0])
    ]


def run(x, W_attn, b_attn, W_proj, b_proj, trace=False, **spmd_kwargs):
    nc = _get_module()
    in_maps = make_in_maps(x, W_attn, b_attn, W_proj, b_proj)
    res = run_bass_kernel_spmd(nc, in_maps, list(range(NCORES)), trace=trace,
                               **spmd_kwargs)
    out = np.stack([res.results[b]["out"] for b in range(len(in_maps))], axis=0)
    return out, res


def kernel(x, W_attn, b_attn, W_proj, b_proj):
    out, _ = run(x, W_attn, b_attn, W_proj, b_proj)
    return out


# revision 16
# speedup vs baseline: 1.0017x; 1.0017x over previous
"""Causal self-attention Trainium2 Bass kernel (V4).

Full-input contract: kernel(**inputs) takes the unsharded inputs
(x [8,1024,768], W_attn [768,2304], b_attn [2304], W_proj [768,768],
b_proj [768]) and returns the full output [8,1024,768].

Sharding: data parallel - batch element b runs on NeuronCore b (B=8 =
n_cores), no collectives needed.

V4 changes vs V3 (trace-driven; V3 span 303us, PE 64% cov, ACT-bound
attention with a 6.5us single-partition reciprocal on the critical path):
  - scores land in bf16 PSUM tiles (1 bank instead of 2): per-head sp
    double-buffered + both heads' score MMs adjacent -> row-group
    concurrent (K=64 pairs at rows 0:64 / 64:128), single MM per tk-tile
    (N up to 1024 bf16 moving).
  - avp split into L/R [65,512] banks; L evicts at i=3 overlapping the
    R-half AV stream. PSUM: 4 sp banks + 4 avp banks = 8 exactly.
  - softmax divide: reciprocal_approx_fast straight off the PSUM l-row,
    1/l broadcast via gpsimd partition_broadcast (SBUF only), multiply
    fused into the avp eviction. No PE broadcast MMs, no yS staging copy.
  - bias adds: b_attn(v part) / b_proj partition-broadcast once via
    gpsimd; evictions become DVE tensor_tensor adds. Kills 32 K=1 bias
    MMs (the qk bias stays a per-partition tensor_scalar_add).
  - pb (exp scores) bf16: halves gpsimd affine_select + AV moving SBUF.
"""

import os
import sys

import numpy as np

for _p in ("/opt/trn_rl_repo", "/root/.axon_site/_ro/trn_rl_repo"):
    if os.path.isdir(_p) and _p not in sys.path:
        sys.path.insert(0, _p)
        break

import concourse.bass as bass
import concourse.mybir as mybir
import concourse.tile as tile
from concourse.bass_utils import run_bass_kernel_spmd
from concourse.masks import make_identity

T, C, H = 1024, 768, 12
C3 = 3 * C
NCORES = 8
NT = T // 128    # 8 t-tiles
NC_ = C // 128   # 6 c-tiles
NM = 2 * C // 128  # 12 m-tiles covering q,k output cols
f32 = mybir.dt.float32
f32r = mybir.dt.float32r
bf16 = mybir.dt.bfloat16

EXP = mybir.ActivationFunctionType.Exp


def build_module():
    md = f32r
    nc = bass.Bass()
    x_d = nc.dram_tensor("x", [T, C], md, kind="ExternalInput")
    wa_d = nc.dram_tensor("W_attn", [C, C3], md, kind="ExternalInput")
    ba_d = nc.dram_tensor("b_attn", [1, C3], md, kind="ExternalInput")
    wp_d = nc.dram_tensor("W_proj", [C, C], md, kind="ExternalInput")
    bp_d = nc.dram_tensor("b_proj", [1, C], md, kind="ExternalInput")
    out_d = nc.dram_tensor("out", [T, C], f32, kind="ExternalOutput")

    with tile.TileContext(nc) as tc:
        with tc.tile_pool(name="persist", bufs=1) as P0:
            identf = P0.tile([128, 128], f32, name="identf")
            make_identity(nc, identf[:])
            ident = P0.tile([128, 128], md, name="ident")
            nc.vector.tensor_copy(ident[:], identf[:])
            ones_f = P0.tile([128, 128], f32, name="ones_f")
            nc.vector.memset(ones_f[:], 1.0)
            ones_col = P0.tile([128, H], bf16, name="ones_col")
            nc.vector.tensor_copy(ones_col[:], ones_f[:, 0:H])

            qkT = [P0.tile([128, T], bf16, name=f"qkT{m}") for m in range(NM)]
            vA = [P0.tile([128, 65 * H], bf16, name=f"vA{t}") for t in range(NT)]
            yT = [P0.tile([128, T], md, name=f"yT{c}") for c in range(NC_)]
            ba_sb = P0.tile([1, C], md, name="ba_sb")
            bp_sb = P0.tile([1, C], md, name="bp_sb")
            baB = P0.tile([128, C], md, name="baB")   # b_attn v-part bcast
            bpB = P0.tile([128, C], md, name="bpB")   # b_proj bcast
            wpt = [P0.tile([128, C], md, name=f"wp{c}") for c in range(NC_)]
            bqk = [P0.tile([128, 1], f32, name=f"bqk{m}") for m in range(NM)]

            # preload the exp table set while ACT is idle (else the first
            # attention exp pays the ~2.7us ACT_TABLE_LOAD inline)
            warm = P0.tile([1, 16], f32, name="warm")
            nc.scalar.activation(warm[:], ones_f[0:1, 0:16], EXP, scale=0.125)

            # ---- phase 1: x load (FIRST DMAs issued) + transpose ----
            with tc.tile_pool(name="sb1", bufs=3) as SB1:
                xT = [SB1.tile([128, T], md, name=f"xT{c}", tag=f"xT{c}", bufs=1)
                      for c in range(NC_)]
                with tc.tile_pool(name="ps1", bufs=1, space="PSUM") as PS1:
                    for j2 in range(2):
                        trs = [PS1.tile([128, 512], md, tag=f"tr{c}", name=f"tr{c}")
                               for c in range(NC_)]
                        for u in range(4):
                            t = 4 * j2 + u
                            xt = SB1.tile([128, C], md, tag="xt", name="xt")
                            nc.sync.dma_start(out=xt[:], in_=x_d[128 * t:128 * (t + 1), :])
                            for c in range(NC_):
                                nc.tensor.transpose(trs[c][:, 128 * u:128 * (u + 1)],
                                                    xt[:, 128 * c:128 * (c + 1)], ident[:])
                        for c in range(NC_):
                            nc.vector.tensor_copy(xT[c][:, 512 * j2:512 * (j2 + 1)],
                                                  trs[c][:])

                # ---- phase 2: v then q^T/k^T ----
                with tc.tile_pool(name="ps12", bufs=2, space="PSUM") as PS12, \
                     tc.tile_pool(name="sb12", bufs=3) as SB12:
                    # v: stationary xT tiles, moving W_v columns
                    wV = [SB12.tile([128, C], md, name=f"wV{c}", tag=f"wV{c}", bufs=1)
                          for c in range(NC_)]
                    for c in range(NC_):
                        nc.sync.dma_start(out=wV[c][:],
                                          in_=wa_d[128 * c:128 * (c + 1), 2 * C:3 * C])
                    nc.sync.dma_start(out=ba_sb[:], in_=ba_d[0:1, 2 * C:3 * C])
                    # one-time bias broadcast (free-dim stride-0 DMA
                    # replicate); scalar-queue-issued: ACT is idle here and
                    # this keeps the sync queue free for weight loads
                    nc.scalar.dma_start(
                        out=baB[:],
                        in_=ba_sb[0:1, :].unsqueeze(1).to_broadcast([1, 128, C]))
                    baB_r = baB.rearrange("p (h e) -> p h e", h=H)
                    for t in range(NT):
                        accv = PS12.tile([128, C], f32, tag="v", name="accv")
                        for c in range(NC_):
                            xcol = xT[c][:, 128 * t:128 * (t + 1)]
                            nc.tensor.matmul(accv[:, 0:512], xcol, wV[c][:, 0:512],
                                             start=(c == 0), stop=(c == NC_ - 1))
                            nc.tensor.matmul(accv[:, 512:C], xcol, wV[c][:, 512:C],
                                             start=(c == 0), stop=(c == NC_ - 1))
                        av = vA[t].rearrange("p (h e) -> p h e", h=H)
                        nc.vector.tensor_copy(
                            av[:, :, 64:65],
                            ones_col.rearrange("p (h o) -> p h o", o=1))
                        # eviction with fused bias add (replaces bias MMs)
                        nc.vector.tensor_tensor(
                            av[:, :, 0:64],
                            accv[:].rearrange("p (h e) -> p h e", h=H),
                            baB_r[:, :, 0:64],
                            mybir.AluOpType.add)

                    # q^T / k^T: W_attn halves batched per c-tile; the k half
                    # reuses the wV slots (freed once the v matmuls finish).
                    # bqk partition-scatter DMAs (4B-granular, slow to issue)
                    # go on the scalar HWDGE queue, off the weight-load path
                    for m in range(NM):
                        nc.scalar.dma_start(
                            out=bqk[m][:],
                            in_=ba_d.bitcast(f32)[0:1, 128 * m:128 * (m + 1)]
                                .rearrange("a p -> p a"))
                    wAq = [SB12.tile([128, C], md, name=f"wAq{c}", tag=f"wAq{c}",
                                     bufs=1) for c in range(NC_)]
                    for half in range(2):
                        if half == 0:
                            wh = wAq
                        else:
                            wh = [SB12.tile([128, C], md, name=f"wAk{c}",
                                            tag=f"wV{c}", bufs=1)
                                  for c in range(NC_)]
                        for c in range(NC_):
                            nc.sync.dma_start(
                                out=wh[c][:],
                                in_=wa_d[128 * c:128 * (c + 1), C * half:C * (half + 1)])
                        for mm in range(NC_):
                            m = NC_ * half + mm
                            acc = PS12.tile([128, T], f32, tag="qk", name="acc")
                            for c in range(NC_):
                                wa = wh[c][:, 128 * mm:128 * (mm + 1)]
                                for j2 in range(2):
                                    nc.tensor.matmul(
                                        acc[:, 512 * j2:512 * (j2 + 1)],
                                        wa,
                                        xT[c][:, 512 * j2:512 * (j2 + 1)],
                                        start=(c == 0), stop=(c == NC_ - 1),
                                    )
                            # psum -> sbuf(bf16) with per-partition bias add
                            nc.vector.tensor_scalar_add(qkT[m][:], acc[:], bqk[m][:])

            # ---- phase 3: attention (head pairs; pipelined scores->AV) ----
            # W_proj / b_proj loads issued here: sync queue is idle now and
            # phase 4 needs them much later
            nc.sync.dma_start(out=bp_sb[:], in_=bp_d[:])
            for c in range(NC_):
                nc.sync.dma_start(out=wpt[c][:], in_=wp_d[128 * c:128 * (c + 1), :])
            nc.scalar.dma_start(
                out=bpB[:],
                in_=bp_sb[0:1, :].unsqueeze(1).to_broadcast([1, 128, C]))
            with tc.tile_pool(name="ps3", bufs=1, space="PSUM") as PS3, \
                 tc.tile_pool(name="sb3", bufs=4) as SB3:
                for hp in range(H // 2):
                    qt = qkT[hp]
                    kt = qkT[NC_ + hp]

                    # per-head-pair state
                    sps = {}        # (i, hs) -> score tile (live window)
                    avs = {}        # (hs, half) -> [65,512] accumulator
                    for hs in range(2):
                        for half in range(2):
                            avs[(hs, half)] = PS3.tile(
                                [65, 512], f32, tag=f"av{hs}{half}", bufs=1,
                                name=f"av{hs}{half}")

                    def emit_score(i):
                        lo = 128 * i
                        for hs in range(2):
                            base = 64 * hs
                            sp = PS3.tile([128, T], f32, tag="s", bufs=2,
                                          name=f"sp{hs}")
                            ktile = kt[base:base + 64, lo:lo + 128]
                            if lo < 512:
                                nc.tensor.matmul(sp[:, lo:512], ktile,
                                                 qt[base:base + 64, lo:512],
                                                 start=True, stop=True)
                                nc.tensor.matmul(sp[:, 512:T], ktile,
                                                 qt[base:base + 64, 512:T],
                                                 start=True, stop=True)
                            else:
                                nc.tensor.matmul(sp[:, lo:T], ktile,
                                                 qt[base:base + 64, lo:T],
                                                 start=True, stop=True)
                            sps[(i, hs)] = sp

                    def emit_exp_sel(i):
                        lo = 128 * i
                        for hs in range(2):
                            sp = sps.pop((i, hs))
                            pb = SB3.tile([128, T], bf16, tag="pb", bufs=6,
                                          name="pb")
                            nc.scalar.activation(pb[:, lo:T], sp[:, lo:T], EXP,
                                                 scale=0.125)
                            # diagonal [128,128] sub-tile: keep iff p <= f
                            nc.gpsimd.affine_select(
                                out=pb[:, lo:lo + 128], in_=pb[:, lo:lo + 128],
                                pattern=[[1, 128]],
                                compare_op=mybir.AluOpType.is_ge, fill=0.0,
                                base=0, channel_multiplier=-1,
                            )
                            sps[("pb", i, hs)] = pb

                    def emit_av(i):
                        lo = 128 * i
                        for hs in range(2):
                            h = 2 * hp + hs
                            pb = sps.pop(("pb", i, hs))
                            vt = vA[i][:, 65 * h:65 * h + 65]
                            if lo < 512:
                                nc.tensor.matmul(avs[(hs, 0)][0:65, lo:512], vt,
                                                 pb[:, lo:512], start=(i == 0),
                                                 stop=(i == 3),
                                                 skip_group_check=True)
                                nc.tensor.matmul(avs[(hs, 1)][0:65, 0:512], vt,
                                                 pb[:, 512:T], start=(i == 0),
                                                 stop=(i == NT - 1),
                                                 skip_group_check=True)
                            else:
                                nc.tensor.matmul(avs[(hs, 1)][0:65, lo - 512:512],
                                                 vt, pb[:, lo:T], start=False,
                                                 stop=(i == NT - 1),
                                                 skip_group_check=True)

                    def emit_norm(half):
                        # Stage the finished accumulator straight to SBUF -
                        # this frees the PSUM bank ~0.7us after the last AV
                        # matmul so the next head-pair never stalls. The
                        # reciprocal/broadcast/multiply then run from SBUF
                        # with the (distant) proj deadline.
                        yU = {}
                        for hs in range(2):
                            yU[hs] = SB3.tile([65, 512], f32, tag=f"yU{hs}",
                                              bufs=3, name=f"yU{hs}")
                            nc.vector.tensor_copy(yU[hs][:],
                                                  avs[(hs, half)][0:65, 0:512])
                        lp = SB3.tile([33, 512], f32, tag="lp", bufs=2,
                                      name="lp")
                        nc.gpsimd.tensor_copy(lp[0:1, :], yU[0][64:65, :])
                        nc.gpsimd.tensor_copy(lp[32:33, :], yU[1][64:65, :])
                        rl = SB3.tile([33, 512], f32, tag="rl", bufs=2,
                                      name="rl")
                        # rows 1..31 are don't-care garbage; only 0 and 32
                        # carry the two heads' l
                        nc.vector.reciprocal(rl[:], lp[:])
                        for hs in range(2):
                            base = 64 * hs
                            rlb = SB3.tile([64, 512], f32, tag="rlb", bufs=4,
                                           name="rlb")
                            nc.sync.dma_start(
                                out=rlb[:],
                                in_=rl[32 * hs:32 * hs + 1, :].unsqueeze(1)
                                    .to_broadcast([1, 64, 512]))
                            nc.vector.tensor_tensor(
                                yT[hp][base:base + 64, 512 * half:512 * (half + 1)],
                                yU[hs][0:64, :], rlb[:],
                                mybir.AluOpType.mult)

                    # software-pipelined emission: scores run one tk-tile
                    # ahead of the AV stream; L-half normalization overlaps
                    # the i>=4 AV matmuls
                    emit_score(0)
                    emit_exp_sel(0)
                    for i in range(1, NT):
                        emit_score(i)
                        emit_exp_sel(i)
                        emit_av(i - 1)
                        if i == 5:
                            emit_norm(0)
                    emit_av(NT - 1)
                    emit_norm(1)

            # ---- phase 4: out = y^T.T @ W_proj + b_proj ----
            with tc.tile_pool(name="ps4", bufs=2, space="PSUM") as PS4, \
                 tc.tile_pool(name="sb4", bufs=3) as SB4:
                for t in range(NT):
                    acc = PS4.tile([128, C], f32, tag="pj", name="acc")
                    for c in range(NC_):
                        ycol = yT[c][:, 128 * t:128 * (t + 1)]
                        nc.tensor.matmul(acc[:, 0:512], ycol, wpt[c][:, 0:512],
                                         start=(c == 0), stop=(c == NC_ - 1))
                        nc.tensor.matmul(acc[:, 512:C], ycol, wpt[c][:, 512:C],
                                         start=(c == 0), stop=(c == NC_ - 1))
                    ot = SB4.tile([128, C], f32, tag="ot", bufs=3, name="ot")
                    # eviction with fused bias add (replaces bias MMs)
                    nc.vector.tensor_tensor(ot[:], acc[:], bpB[:],
                                            mybir.AluOpType.add)
                    nc.sync.dma_start(out=out_d[128 * t:128 * (t + 1), :], in_=ot[:])

    return nc


_WAIT_SKIP = {"InstNoOp", "InstEventSemOp", "InstSemaphoreOp",
              "InstCustomDveAnt", "InstPartitionBroadcast",
              "InstPartitionAllReduce"}


def _legalize_waits(nc):
    """walrus's codegen allows limited sync-wait commands per ISA struct
    (e.g. a Matmult's waits all land on the generated LDWEIGHTS struct which
    has one slot). Move excess waits onto same-engine NoOps inserted
    immediately before the instruction - program order on the engine queue
    preserves the synchronization semantics."""
    nfix = 0
    for fn in nc.m.functions:
        for bb in fn.blocks:
            out = []
            for ins in bb.instructions:
                si = ins.sync_info
                if (type(ins).__name__ not in _WAIT_SKIP and si is not None
                        and si.on_wait and len(si.on_wait) > 1):
                    waits = list(si.on_wait)
                    extra, keep = waits[:-1], waits[-1:]
                    for k, w in enumerate(extra):
                        nop = mybir.InstNoOp(name=f"{ins.name}-wf{k}", ins=[], outs=[])
                        nop.engine = ins.engine
                        nop.sync_info = mybir.SyncInfo(on_wait=[w], on_update=[])
                        out.append(nop)
                    ins.sync_info = mybir.SyncInfo(
                        on_wait=keep, on_update=list(si.on_update or []))
                    nfix += 1
                out.append(ins)
            bb.instructions = out
    return nfix


_cached_module = None


def _get_module():
    global _cached_module
    if _cached_module is None:
        nc = build_module()
        _legalize_waits(nc)
        _cached_module = nc
    return _cached_module


def make_in_maps(x, W_attn, b_attn, W_proj, b_proj):
    x = np.asarray(x, dtype=np.float32)
    wa = np.ascontiguousarray(np.asarray(W_attn, dtype=np.float32))
    ba = np.ascontiguousarray(np.asarray(b_attn, dtype=np.float32).reshape(1, C3))
    wp = np.ascontiguousarray(np.asarray(W_proj, dtype=np.float32))
    bp = np.ascontiguousarray(np.asarray(b_proj, dtype=np.float32).reshape(1, C))
    return [
        dict(x=np.ascontiguousarray(x[b]), W_attn=wa, b_attn=ba, W_proj=wp, b_proj=bp)
        for b in range(x.shape[0])
    ]


def run(x, W_attn, b_attn, W_proj, b_proj, trace=False, **spmd_kwargs):
    nc = _get_module()
    in_maps = make_in_maps(x, W_attn, b_attn, W_proj, b_proj)
    res = run_bass_kernel_spmd(nc, in_maps, list(range(NCORES)), trace=trace,
                               **spmd_kwargs)
    out = np.stack([res.results[b]["out"] for b in range(len(in_maps))], axis=0)
    return out, res


def kernel(x, W_attn, b_attn, W_proj, b_proj):
    out, _ = run(x, W_attn, b_attn, W_proj, b_proj)
    return out
